# revision 13
# baseline (speedup 1.0000x reference)
"""GaussianImage rasterization on 8 Trainium2 NeuronCores.

Strategy: shard *pixels* (not gaussians). The 256x256 image is divided into
128 tiles of 16x32 px. Each gaussian provably influences only pixels within
5px (alpha < 1/255 beyond -> masked to 0 by the reference), so gaussians are
binned per-tile on the host. Tiles are balance-assigned 16-per-core.

Per tile, gaussians are split into chunks of <=32 ("quarters"). Four quarters
form a "pack" occupying the full 128 SBUF partitions x 512 free (the tile's
pixels). Per pack the device does:
  sigma  = U[6,128]^T @ V[6,512]          (TensorE, quadratic-form expansion)
  e      = exp(-sigma + ln(opacity))      (ScalarE, bias per partition)
  w      = (e >= 1/255) * e               (VectorE, one fused op)
  img[3,512] += F[32,3]^T @ w[32,512]     (TensorE x4, row/col tile_position,
                                           accumulating in PSUM per image tile)
Image tiles live in 4 PSUM banks (4 tiles/bank at partition offsets 0/32/64/96)
and are DMA'd straight to DRAM at the end. No collectives needed - pixel
shards are disjoint; the host assembles and clips.

All per-gaussian math (tanh/sigmoid/conic/expansion coefficients) is done on
the host in float64: it is O(N)=2048 work vs the O(N*H*W) rasterization.
Tile-local centered coordinates keep the quadratic expansion's terms small so
f32 (and the f32r hi/lo split) stays accurate.
"""

import functools
import math
import os

import numpy as np

H = W = 256
TH, TW = 16, 32          # tile shape
NTR, NTC = H // TH, W // TW   # 16 x 8 = 128 tiles
NCORES = 8
TILES_PER_CORE = (NTR * NTC) // NCORES  # 16
QCAP = 32                # gaussians per quarter
FREE = TH * TW           # 512 pixels per tile
ALPHA_MIN = 1.0 / 255.0
BIG_SIGMA = 1.0e9

# "f32": true-fp32 sigma matmul (4 PE passes). "f32r_hilo": two f32r passes
# with a hi/lo mantissa split of U (V is exact in fp22) - same accuracy,
# half the PE cost.
SIGMA_MODE = os.environ.get("GS_SIGMA_MODE", "f32")

LAST_EXEC_TIME_NS = None
LAST_RESULTS = None


def _trunc_fp22(x):
    xi = np.ascontiguousarray(np.asarray(x, np.float32)).view(np.uint32)
    return (xi & np.uint32(0xFFFFFC00)).view(np.float32)


def _project(xyz, scaling, rotation, opacity):
    """Reference activations + projection, in float64 on host (O(N) work)."""
    xyz = xyz.astype(np.float64)
    scaling = scaling.astype(np.float64)
    rotation = rotation.astype(np.float64)
    op = opacity.astype(np.float64)[:, 0]
    xy = np.tanh(xyz)
    scale = np.abs(scaling + 0.5)
    theta = (1.0 / (1.0 + np.exp(-rotation[:, 0]))) * (2.0 * math.pi)
    cx = 0.5 * ((xy[:, 0] + 1.0) * W - 1.0)
    cy = 0.5 * ((xy[:, 1] + 1.0) * H - 1.0)
    c, s = np.cos(theta), np.sin(theta)
    sx2, sy2 = scale[:, 0] ** 2, scale[:, 1] ** 2
    cov_a = c * c * sx2 + s * s * sy2
    cov_b = c * s * (sx2 - sy2)
    cov_d = s * s * sx2 + c * c * sy2
    det = cov_a * cov_d - cov_b * cov_b
    qa, qb, qc = cov_d / det, -cov_b / det, cov_a / det
    # influence radius: alpha = op*exp(-sigma) >= 1/255 requires
    # sigma <= log(255*op); sigma >= |d|^2 / (2*max(sx2,sy2)).
    thr = np.log(255.0 * np.maximum(op, 1e-30))
    radius = np.sqrt(np.maximum(2.0 * np.maximum(sx2, sy2) * thr, 0.0)) + 1e-3
    return dict(cx=cx, cy=cy, qa=qa, qb=qb, qc=qc, op=op, radius=radius)


def _bin_tiles(proj):
    lists = [[] for _ in range(NTR * NTC)]
    cx, cy, r = proj["cx"], proj["cy"], proj["radius"]
    n = cx.shape[0]
    for g in range(n):
        r0 = max(0, int(math.floor((cy[g] - r[g]) / TH)))
        r1 = min(NTR - 1, int(math.floor((cy[g] + r[g]) / TH)))
        c0 = max(0, int(math.floor((cx[g] - r[g]) / TW)))
        c1 = min(NTC - 1, int(math.floor((cx[g] + r[g]) / TW)))
        for tr in range(r0, r1 + 1):
            for tc in range(c0, c1 + 1):
                lists[tr * NTC + tc].append(g)
    return lists


def _assign_tiles(lists):
    """Balance tiles across cores by quarter count; 16 tiles per core.
    Returns per-core list of tile ids (sorted desc by quarters) and the
    unified per-position quarter profile."""
    nq = [max(1, (len(l) + QCAP - 1) // QCAP) for l in lists]
    order = sorted(range(len(lists)), key=lambda t: -nq[t])
    totals = [0] * NCORES
    core_tiles = [[] for _ in range(NCORES)]
    for t in order:
        cands = [c for c in range(NCORES) if len(core_tiles[c]) < TILES_PER_CORE]
        c = min(cands, key=lambda c: (totals[c], len(core_tiles[c])))
        core_tiles[c].append(t)
        totals[c] += nq[t]
    # per-position unified quarter counts (tiles are already desc within core)
    profile = [max(nq[core_tiles[c][pos]] for c in range(NCORES))
               for pos in range(TILES_PER_CORE)]
    return core_tiles, profile


def _build_V():
    py = np.arange(TH, dtype=np.float64) - (TH - 1) / 2.0
    px = np.arange(TW, dtype=np.float64) - (TW - 1) / 2.0
    PY, PX = np.meshgrid(py, px, indexing="ij")
    PX, PY = PX.ravel(), PY.ravel()
    V = np.stack([np.ones_like(PX), PX, PY, PX * PX, PX * PY, PY * PY])
    return V.astype(np.float32)


def _quarter_desc(profile):
    """Flat (pos,) per quarter, with each img-bank's quarter run padded to a
    multiple of 4 so a pack maps to exactly one bank. A pack does ONE K=128
    feature matmul into its bank's [12,512] PSUM region: each quarter's F
    occupies its own 32 rows x its position's 3 columns, zeros elsewhere
    contribute exact +0. Returns (seq, pack_start, pack_stop): per-pack
    accumulation flags (first/last pack of the bank)."""
    seq = []
    for b in range(4):
        for pos in range(4 * b, 4 * b + 4):
            seq.extend([pos] * profile[pos])
        while len(seq) % 4:
            seq.append(4 * b)  # dummy quarter, same bank
    pack_bank = [seq[4 * p] // 4 for p in range(len(seq) // 4)]
    pack_start = [pack_bank.index(b) == p for p, b in enumerate(pack_bank)]
    pack_stop = [len(pack_bank) - 1 - pack_bank[::-1].index(b) == p
                 for p, b in enumerate(pack_bank)]
    return tuple(seq), tuple(pack_start), tuple(pack_stop)


def _build_core_data(core_tiles_c, lists, proj, features, desc, npack):
    """U (or Uhi/Ulo), F, B arrays for one core."""
    sigma_hilo = SIGMA_MODE == "f32r_hilo"
    U = np.zeros((6, npack * 128), np.float64)
    U[0, :] = BIG_SIGMA                       # default: dummy gaussian
    F = np.zeros((128, npack * 12), np.float32)
    B = np.zeros((128, npack), np.float32)
    for k, pos in enumerate(desc):
        p, j = k // 4, k % 4
        t = core_tiles_c[pos]
        glist = lists[t]
        # chunk j' of this tile: which chunk does quarter k represent?
        # quarters of one pos are consecutive in desc; find j' = index within pos run
        # (desc was built pos-major so count previous quarters of same pos)
        jprime = sum(1 for kk in range(k) if desc[kk] == pos)
        chunk = glist[jprime * QCAP:(jprime + 1) * QCAP]
        if not chunk:
            continue
        g = np.asarray(chunk, dtype=int)
        tr, tc = t // NTC, t % NTC
        oy = TH * tr + (TH - 1) / 2.0
        ox = TW * tc + (TW - 1) / 2.0
        cxl = proj["cx"][g] - ox
        cyl = proj["cy"][g] - oy
        qa, qb, qc = proj["qa"][g], proj["qb"][g], proj["qc"][g]
        cols = slice(p * 128 + 32 * j, p * 128 + 32 * j + len(g))
        U[0, cols] = 0.5 * qa * cxl * cxl + qb * cxl * cyl + 0.5 * qc * cyl * cyl
        U[1, cols] = -(qa * cxl + qb * cyl)
        U[2, cols] = -(qb * cxl + qc * cyl)
        U[3, cols] = 0.5 * qa
        U[4, cols] = qb
        U[5, cols] = 0.5 * qc
        fc = 12 * p + 3 * (pos % 4)
        F[32 * j:32 * j + len(g), fc:fc + 3] = features[g].astype(np.float32)
        B[32 * j:32 * j + len(g), p] = np.log(
            np.maximum(proj["op"][g], 1e-30)).astype(np.float32)
    U32 = U.astype(np.float32)
    FB = np.concatenate([F, B], axis=1)
    if sigma_hilo:
        Uhi = _trunc_fp22(U32)
        Ulo = (U32 - Uhi).astype(np.float32)
        UV = np.concatenate([Uhi, Ulo, _build_V()], axis=1)
    else:
        UV = np.concatenate([U32, _build_V()], axis=1)
    return {"uv_in": UV, "fb_in": FB}


# desc is hashable; program structure depends only on it + sigma mode
@functools.lru_cache(maxsize=4)
def _build_program(desc, pack_start, pack_stop, sigma_mode, repeat=1):
    import concourse.bacc as bacc
    import concourse.tile as tile
    from concourse import mybir

    npack = len(desc) // 4
    f32 = mybir.dt.float32
    f32r = mybir.dt.float32r
    nc = bacc.Bacc("TRN2", target_bir_lowering=False, debug=False,
                   num_devices=NCORES)
    hilo = sigma_mode == "f32r_hilo"
    nu = 2 if hilo else 1                    # U passes (hi/lo or single)
    uv_dt = f32r if hilo else f32            # true-f32 MM needs f32 operands
    uvw = nu * npack * 128 + FREE            # uhi | [ulo] | v, on 6 partitions
    UV_d = nc.dram_tensor("uv_in", [6, uvw], uv_dt, kind="ExternalInput").ap()
    FB_d = nc.dram_tensor("fb_in", [128, npack * 13], f32r,
                          kind="ExternalInput").ap()
    out_d = nc.dram_tensor("img_out", [4, 12, FREE], f32,
                           kind="ExternalOutput").ap()

    with tile.TileContext(nc) as tc:
        with tc.tile_pool(name="const", bufs=1) as cpool, \
             tc.tile_pool(name="sig", bufs=2, space="PSUM") as sig_pool, \
             tc.tile_pool(name="img", bufs=1, space="PSUM") as img_pool, \
             tc.tile_pool(name="work", bufs=3) as wpool:
            UV_sb = cpool.tile([6, uvw], uv_dt, tag="uv", name="uv_sb")
            nc.sync.dma_start(out=UV_sb[:, :], in_=UV_d)
            FB_sb = cpool.tile([128, npack * 13], f32r, tag="fb", name="fb_sb")
            nc.sync.dma_start(out=FB_sb[:, :], in_=FB_d)
            nuv = nu * npack * 128
            V_sb = UV_sb[:, nuv:nuv + FREE]
            F_sb = FB_sb[:, :npack * 12]
            B_sb = FB_sb[:, npack * 12:].bitcast(f32)

            img_banks = [img_pool.tile([12, FREE], f32, tag=f"img{b}", name=f"img{b}")
                         for b in range(4)]

            for rep in range(repeat):
              for p in range(npack):
                sig = sig_pool.tile([128, FREE], f32, tag="sig",
                                    name=f"sig{rep}_{p}")
                for iu in range(nu):
                    off = iu * npack * 128 + 128 * p
                    nc.tensor.matmul(
                        sig[:, :], UV_sb[:, off:off + 128], V_sb[:, :],
                        start=(iu == 0), stop=(iu == nu - 1),
                        skip_group_check=True)
                e = wpool.tile([128, FREE], f32, tag="e", name=f"e{rep}_{p}")
                nc.scalar.activation(
                    e[:, :], sig[:, :], mybir.ActivationFunctionType.Exp,
                    bias=B_sb[:, p:p + 1], scale=-1.0)
                wt = wpool.tile([128, FREE], f32r, tag="w", name=f"w{rep}_{p}")
                nc.vector.scalar_tensor_tensor(
                    wt[:, :], e[:, :], float(ALPHA_MIN), e[:, :],
                    mybir.AluOpType.is_ge, mybir.AluOpType.mult)
                bank = desc[4 * p] // 4
                nc.tensor.matmul(
                    img_banks[bank][0:12, :],
                    F_sb[0:128, 12 * p:12 * p + 12],
                    wt[0:128, :],
                    start=pack_start[p], stop=pack_stop[p],
                    skip_group_check=True)

            for b in range(4):
                ob = wpool.tile([12, FREE], f32, tag=f"ob{b}", name=f"ob{b}",
                                bufs=1)
                if b % 2 == 0:
                    nc.vector.tensor_copy(ob[:, :], img_banks[b][:, :])
                else:
                    nc.scalar.copy(ob[:, :], img_banks[b][:, :])
                nc.sync.dma_start(out=out_d[b], in_=ob[:, :])
    nc.compile()
    return nc


def kernel(xyz, scaling, rotation, features, opacity):
    global LAST_EXEC_TIME_NS, LAST_RESULTS
    from concourse.bass_utils import run_bass_kernel_spmd

    proj = _project(xyz, scaling, rotation, opacity)
    lists = _bin_tiles(proj)
    core_tiles, profile = _assign_tiles(lists)
    desc, pack_start, pack_stop = _quarter_desc(profile)
    npack = len(desc) // 4

    in_maps = []
    for c in range(NCORES):
        m = _build_core_data(core_tiles[c], lists, proj,
                             np.asarray(features), desc, npack)
        in_maps.append(m)

    nc = _build_program(desc, pack_start, pack_stop, SIGMA_MODE)
    trace = os.environ.get("GS_TRACE", "0") == "1"
    res = run_bass_kernel_spmd(nc, in_maps, core_ids=list(range(NCORES)),
                               trace=trace)
    LAST_EXEC_TIME_NS = res.exec_time_ns
    LAST_RESULTS = res

    img = np.zeros((3, H, W), np.float32)
    for c in range(NCORES):
        out = res.results[c]["img_out"].reshape(4, 4, 3, FREE)
        for pos in range(TILES_PER_CORE):
            t = core_tiles[c][pos]
            tr, tc = t // NTC, t % NTC
            img[:, TH * tr:TH * tr + TH, TW * tc:TW * tc + TW] = \
                out[pos // 4, pos % 4].reshape(3, TH, TW)
    img = np.clip(img, 0.0, 1.0)
    return img[None].astype(np.float32)
